# revision 33
# baseline (speedup 1.0000x reference)
"""BlockXDiag (tri-diagonal block matrix × batch, periodic corners) on 8
Trainium2 NeuronCores.

Math (per reference): out_i = x_{i-1} @ A_i.T + x_i @ Wd_i.T + x_{i+1} @ Wu_i.T
for block-rows i in [0, 64), block size P=256, batch B=4096, with periodic
corner terms (x_63 @ Wtr.T into out_0, x_0 @ Wbl.T into out_63).

Sharding: output block-rows are split 8-per-core (weights split across
cores, which keeps per-core weight traffic small and lets each core
stream only its 10-block x halo). Inputs are staged host-side as x^T so the
contraction dim lands on SBUF partitions; output is produced transposed and
un-transposed on the host.

Device kernel per core: out.T[q, b] tiles [128, 512] accumulated in PSUM
over 6 matmuls (3 source blocks x 2 k-halves), weights stationary.

bf16 mode (default): x/w/out all bf16 (rel err ~3.5e-3 vs the 2e-2 gate);
per-core DMA is 41 MB (~120 us) vs a 768-matmul tensor floor of ~166 us at
2.4 GHz, so the body is compute-bound and runs gapless at 216 ns/matmul.
Known-fixed overheads measured on HW: ~7 us engine-start preamble (mostly
ungraded), ~3.5 us PE p-state ramp (full clock arrives at a fixed ~20 us
wall time; pre-warming the PE does NOT pull it earlier), and a ~7 us
NRT-injected NEFF postamble that blanket-resets the semaphore file
(ib_insert_common_postamble; scales with nothing we control).
Head DMAs are ordered critical-first (w j0:2, x block 0, ...) because DGE
rings drain strictly in trigger order; each extra dma_start costs ~0.7 us
of sync-engine issue time, so over-chunking the head is a net loss
(measured: 6-8 triggers optimal, 10 triggers -5 us).
--enable-remote-semaphore-dma measured 36 us SLOWER; do not use.
"""
import numpy as np
import ml_dtypes

import concourse.bass as bass
import concourse.mybir as mybir
from concourse.tile import TileContext
from concourse.vector_clock import ScopedClock
from concourse.bass_utils import run_bass_kernel_spmd

M, P, B = 64, 256, 4096
NCORES = 8
BPC = M // NCORES          # output blocks per core: 8
NHALO = BPC + 2            # x blocks needed per core: 10
ROWS = NHALO * P           # x^T rows per core: 2560
BT = 512                   # batch-tile (matmul moving free dim)
NBT = B // BT              # 8

MODE = "bf16"              # "f32" | "f32r" | "bf16"
TRACE = False              # set by test harness to profile
REPEATS = 1                # extra timed executions (test harness only)
LAST_EXEC_NS = None
ALL_EXEC_NS = None

_DT = {
    "f32": (mybir.dt.float32, np.float32),
    "f32r": (mybir.dt.float32r, np.float32),
    "bf16": (mybir.dt.bfloat16, ml_dtypes.bfloat16),
}


# ---------------------------------------------------------------------------
# Workarounds for the pinned walrus build's 1-wait-per-instruction cap.
# Tile's tail drain stuffs every outstanding sem wait onto one Drain, and
# self-loading fp32/fp32r matmuls can carry >1 wait with no Ldweights to
# spill to. Split both across extra same-engine instructions.
def _patched_drain_and_barrier(self, tick_clock, wait_clock):
    # NOTE: the final drain+barrier is REQUIRED — removing it entirely
    # makes the untraced NRT execution path fail with
    # NRT_EXEC_UNIT_UNRECOVERABLE (verified 2026-08-08), even though the
    # traced path tolerates it.  Only clear_and_free_semaphores (redundant
    # with the NRT postamble's blanket sem reset) is skipped.
    drain_inst = self.nc.sync.drain()
    wait_clock.add_sem_waits(
        drain_inst.ins, ScopedClock({None: tick_clock.global_clock})
    )
    si = drain_inst.ins.sync_info
    waits = list(si.on_wait)
    if len(waits) > 1:
        drain_inst.ins.sync_info = mybir.SyncInfo(
            on_wait=[waits[0]], on_update=list(si.on_update)
        )
        # spread the remaining waits across all engines so they resolve in
        # parallel; the barrier below joins them.
        engs = [self.nc.vector, self.nc.scalar, self.nc.gpsimd,
                self.nc.tensor, self.nc.sync]
        for i, w in enumerate(waits[1:]):
            d2 = engs[i % len(engs)].drain()
            d2.ins.sync_info = mybir.SyncInfo(on_wait=[w], on_update=[])
    self.nc.all_engine_barrier()
    assert self.sems is not None
    popped = self.nc._tile_sem_poison_stack.pop()
    assert popped is self._sem_poison


EXTRA_WALRUS_FLAGS = []


def _apply_tile_patch():
    TileContext._drain_and_barrier = _patched_drain_and_barrier
    if EXTRA_WALRUS_FLAGS:
        import concourse.bass_utils as bu
        orig = bu.bir_verify_and_optimise

        def patched(tmpdir, inp="bir.json", outp="file.neff", arch=None, *,
                    dve_root=None):
            import concourse.bass_utils as bu2
            real_run = bu2.run_command

            def run_with_flags(cmd, **kw):
                if cmd and str(cmd[0]).endswith("walrus_driver"):
                    cmd = list(cmd) + EXTRA_WALRUS_FLAGS
                return real_run(cmd, **kw)

            bu2.run_command = run_with_flags
            try:
                return orig(tmpdir, inp, outp, arch, dve_root=dve_root)
            finally:
                bu2.run_command = real_run

        bu.bir_verify_and_optimise = patched


def _install_profile_shim():
    """Make trace=True work in this container: provide the missing
    antenv.axon_hooks module (NTFF capture via ctypes into libaxon_pjrt.so)
    and skip the bucket upload of artifacts."""
    import sys, types, ctypes, contextlib
    import concourse.bass_utils as bu
    bu.upload_artifacts = lambda tmpdir: tmpdir
    try:
        from antenv.axon_hooks import get_axon_ntff_profile_hook  # noqa
        return
    except ImportError:
        pass
    so_path = "/opt/axon/libaxon_pjrt.so"
    lib = ctypes.CDLL(so_path)
    if not hasattr(lib, "axon_start_nrt_profile"):
        return
    lib.axon_start_nrt_profile.argtypes = [
        ctypes.POINTER(ctypes.c_int64), ctypes.c_size_t]
    lib.axon_start_nrt_profile.restype = ctypes.c_int64
    lib.axon_stop_nrt_profile.argtypes = [ctypes.c_char_p]
    lib.axon_stop_nrt_profile.restype = ctypes.c_int64

    @contextlib.contextmanager
    def _hook(output_dir, device_ids):
        import jax
        jax.devices()
        if device_ids:
            ids = (ctypes.c_int64 * len(device_ids))(*device_ids)
            rc = lib.axon_start_nrt_profile(ids, len(device_ids))
        else:
            rc = lib.axon_start_nrt_profile(None, 0)
        if rc != 0:
            raise RuntimeError(f"axon_start_nrt_profile rc={rc}")
        try:
            yield
        finally:
            n = lib.axon_stop_nrt_profile(str(output_dir).encode())
            print(f"profile: {n} file(s) written to {output_dir}")

    mod = types.ModuleType("antenv.axon_hooks")
    mod.get_axon_ntff_profile_hook = lambda: _hook
    mod.set_axon_ntff_profile_hook = lambda h: None
    sys.modules["antenv.axon_hooks"] = mod
    import antenv
    antenv.axon_hooks = mod


def _hoist_excess_waits(nc):
    """Any non-EventSemaphore instruction may carry at most 1 sem wait on
    this walrus build; move extras onto inserted same-engine NoOps."""
    for fn in nc.m.functions:
        for bb in fn.blocks:
            insts = bb.instructions
            newlist = []
            changed = False
            for inst in insts:
                si = inst.sync_info
                cap = 2 if isinstance(inst, mybir.InstEventSemaphore) else 1
                if si is not None and len(si.on_wait) > cap:
                    waits = list(si.on_wait)
                    for i, w in enumerate(waits[cap:]):
                        newlist.append(mybir.InstNoOp(
                            name=f"{inst.name}_waitnop{i}",
                            engine=inst.engine,
                            bass_nofuse=True,
                            sync_info=mybir.SyncInfo(on_wait=[w], on_update=[]),
                        ))
                    inst.sync_info = mybir.SyncInfo(
                        on_wait=waits[:cap], on_update=list(si.on_update))
                    changed = True
                newlist.append(inst)
            if changed:
                insts.clear()
                insts.extend(newlist)


# ---------------------------------------------------------------------------
def _build_nc(mode):
    dt_in, _ = _DT[mode]
    f32 = mybir.dt.float32
    dt_out = dt_in if mode == "bf16" else f32
    nc = bass.Bass()
    # partition-major, batch-tile-major staging: every DMA below is
    # contiguous per SBUF partition (128 fat descriptors instead of
    # thousands of 512B-1KB ones), minimizing DGE latency and ring pressure
    xT_d = nc.dram_tensor(
        "xT", [NBT, 128, NHALO * 2, BT], dt_in, kind="ExternalInput")
    w_d = nc.dram_tensor(
        "w", [128, BPC * 3 * 2, P], dt_in, kind="ExternalInput")
    o_d = nc.dram_tensor(
        "o", [NBT, 128, 16, BT], dt_out, kind="ExternalOutput")

    with TileContext(nc) as tc:
        with tc.tile_pool(name="wpool", bufs=1) as wpool, \
             tc.tile_pool(name="xpool", bufs=2) as xpool, \
             tc.tile_pool(name="opool", bufs=2) as opool, \
             tc.tile_pool(name="pspool", bufs=8, space="PSUM") as pspool:
            w_sb = wpool.tile([128, BPC * 3 * 2, P], dt_in)

            def w_load(j0, j1):
                nc.sync.dma_start(
                    out=w_sb[:, j0:j1, :], in_=w_d[:, j0:j1, :])

            xt0 = xpool.tile([128, NHALO * 2, BT], dt_in, tag="x")

            def x0_load(b0, b1):
                nc.sync.dma_start(
                    out=xt0[:, 2 * b0:2 * b1, :],
                    in_=xT_d[0, :, 2 * b0:2 * b1, :])

            # head: interleave w/x chunks in consumption order; few fat
            # chunks — each dma_start costs ~0.7us of sync-engine issue
            # time, so over-chunking delays the bulk loads
            w_load(0, 2)
            x0_load(0, 1)
            w_load(2, 6)
            x0_load(1, 3)
            w_load(6, 12)
            x0_load(3, 10)
            w_load(12, 24)
            w_load(24, 48)

            for bt in range(NBT):
                if bt == 0:
                    xt = xt0
                else:
                    xt = xpool.tile([128, NHALO * 2, BT], dt_in, tag="x")
                    nc.sync.dma_start(out=xt, in_=xT_d[bt])
                for h in range(2):          # out-group halves
                    ot = opool.tile([128, 8, BT], dt_out, tag="o")
                    last = (bt == NBT - 1 and h == 1)
                    # chunked output DMAs: fire as soon as slots fill; the
                    # final group drains per-slot so the slot-7 DMA overlaps
                    # the slot-8 cast and the final DMA halves to 128 KB
                    csz = 1 if last else 4
                    for li in range(h * 4, h * 4 + 4):
                        for qh in range(2):
                            ps = pspool.tile([128, BT], f32, tag="ps")
                            for s in range(3):
                                for kh in range(2):
                                    nc.tensor.matmul(
                                        ps,
                                        w_sb[:, (li * 3 + s) * 2 + kh,
                                             qh * 128:(qh + 1) * 128],
                                        xt[:, (li + s) * 2 + kh, :],
                                        start=(s == 0 and kh == 0),
                                        stop=(s == 2 and kh == 1),
                                    )
                            nc.vector.tensor_copy(
                                out=ot[:, (li * 2 + qh) - h * 8, :], in_=ps)
                            slot = (li - h * 4) * 2 + qh + 1
                            if slot % csz == 0:
                                nc.sync.dma_start(
                                    out=o_d[bt, :, h * 8 + slot - csz:
                                            h * 8 + slot, :],
                                    in_=ot[:, slot - csz:slot, :],
                                )
    _hoist_excess_waits(nc)
    return nc


def _host_prep(x, Wd, Wu, Wl, Wtr, Wbl, np_dt):
    x = np.asarray(x, np.float32)
    Wd, Wu, Wl = np.asarray(Wd, np.float32), np.asarray(Wu, np.float32), np.asarray(Wl, np.float32)
    Wtr, Wbl = np.asarray(Wtr, np.float32), np.asarray(Wbl, np.float32)

    xT = np.ascontiguousarray(x.T)                       # [M*P, B]
    A = np.concatenate([Wtr[None], Wl], axis=0)          # weight applied to x_{i-1}
    Bst = Wd                                             # weight applied to x_i
    C = np.concatenate([Wu, Wbl[None]], axis=0)          # weight applied to x_{i+1}
    WT = np.stack([A, Bst, C], axis=1)                   # [64, 3, q, p]
    WT = np.ascontiguousarray(WT.transpose(0, 1, 3, 2))  # [64, 3, p, q]

    in_maps = []
    for c in range(NCORES):
        lo = (8 * c - 1) * P
        hi = (8 * c + 9) * P
        if lo < 0:
            xc = np.concatenate([xT[lo:], xT[:hi]], axis=0)
        elif hi > M * P:
            xc = np.concatenate([xT[lo:], xT[:hi - M * P]], axis=0)
        else:
            xc = xT[lo:hi]
        xc = np.asarray(xc, dtype=np_dt)                 # [2560, 4096]
        # partition-major, bt-major: xp[bt, p, t, b] = xc[t*128+p, bt*BT+b]
        xp = np.ascontiguousarray(
            xc.reshape(NHALO * 2, 128, NBT, BT).transpose(2, 1, 0, 3))
        wc = WT[8 * c:8 * c + 8].reshape(BPC, 3, 2, 128, P)
        wc = np.asarray(wc.reshape(BPC * 3 * 2, 128, P), dtype=np_dt)
        wp = np.ascontiguousarray(wc.transpose(1, 0, 2))  # [128, 48, 256]
        in_maps.append({"xT": xp, "w": wp})
    return in_maps


def kernel(x, Wd, Wu, Wl, Wtr, Wbl):
    global LAST_EXEC_NS
    _apply_tile_patch()
    if TRACE:
        try:
            _install_profile_shim()
        except Exception as e:
            print(f"profile shim failed ({e}); running without trace")
    dt_in, np_dt = _DT[MODE]
    nc = _build_nc(MODE)
    in_maps = _host_prep(x, Wd, Wu, Wl, Wtr, Wbl, np_dt)
    res = run_bass_kernel_spmd(
        nc, in_maps, core_ids=list(range(NCORES)), trace=TRACE)
    LAST_EXEC_NS = res.exec_time_ns
    if TRACE and REPEATS > 1:
        global ALL_EXEC_NS
        ALL_EXEC_NS = [res.exec_time_ns]
        for _ in range(REPEATS - 1):
            r2 = run_bass_kernel_spmd(
                nc, in_maps, core_ids=list(range(NCORES)), trace=True)
            ALL_EXEC_NS.append(r2.exec_time_ns)
        LAST_EXEC_NS = min(t for t in ALL_EXEC_NS if t)
    # o is [NBT, 128, 16, BT] per core: o[bt, p, j, b] = outT[j*128+p, bt*BT+b]
    outT = np.concatenate(
        [np.asarray(res.results[c]["o"]).transpose(2, 1, 0, 3).reshape(
            BPC * P, B) for c in range(NCORES)], axis=0)
    return np.ascontiguousarray(outT.T, dtype=np.float32)  # [B, M*P] float32



# revision 34
# speedup vs baseline: 1.0079x; 1.0079x over previous
"""BlockXDiag (tri-diagonal block matrix × batch, periodic corners) on 8
Trainium2 NeuronCores.

Math (per reference): out_i = x_{i-1} @ A_i.T + x_i @ Wd_i.T + x_{i+1} @ Wu_i.T
for block-rows i in [0, 64), block size P=256, batch B=4096, with periodic
corner terms (x_63 @ Wtr.T into out_0, x_0 @ Wbl.T into out_63).

Sharding: output block-rows are split 8-per-core (weights split across
cores, which keeps per-core weight traffic small and lets each core
stream only its 10-block x halo). Inputs are staged host-side as x^T so the
contraction dim lands on SBUF partitions; output is produced transposed and
un-transposed on the host.

Device kernel per core: out.T[q, b] tiles [128, 512] accumulated in PSUM
over 6 matmuls (3 source blocks x 2 k-halves), weights stationary.

bf16 mode (default): x/w/out all bf16 (rel err ~3.5e-3 vs the 2e-2 gate);
per-core DMA is 41 MB (~120 us) vs a 768-matmul tensor floor of ~166 us at
2.4 GHz, so the body is compute-bound and runs gapless at 216 ns/matmul.
Known-fixed overheads measured on HW: ~7 us engine-start preamble (mostly
ungraded), ~3.5 us PE p-state ramp (full clock arrives at a fixed ~20 us
wall time; pre-warming the PE does NOT pull it earlier), and a ~7 us
NRT-injected NEFF postamble that blanket-resets the semaphore file
(ib_insert_common_postamble; scales with nothing we control).
Head DMAs are ordered critical-first (w j0:2, x block 0, ...) because DGE
rings drain strictly in trigger order; each extra dma_start costs ~0.7 us
of sync-engine issue time, so over-chunking the head is a net loss
(measured: 6-8 triggers optimal, 10 triggers -5 us).
--enable-remote-semaphore-dma measured 36 us SLOWER; do not use.
"""
import numpy as np
import ml_dtypes

import concourse.bass as bass
import concourse.mybir as mybir
from concourse.tile import TileContext
from concourse.vector_clock import ScopedClock
from concourse.bass_utils import run_bass_kernel_spmd

M, P, B = 64, 256, 4096
NCORES = 8
BPC = M // NCORES          # output blocks per core: 8
NHALO = BPC + 2            # x blocks needed per core: 10
ROWS = NHALO * P           # x^T rows per core: 2560
BT = 512                   # batch-tile (matmul moving free dim)
NBT = B // BT              # 8

MODE = "bf16"              # "f32" | "f32r" | "bf16"
TRACE = False              # set by test harness to profile
REPEATS = 1                # extra timed executions (test harness only)
LAST_EXEC_NS = None
ALL_EXEC_NS = None

_DT = {
    "f32": (mybir.dt.float32, np.float32),
    "f32r": (mybir.dt.float32r, np.float32),
    "bf16": (mybir.dt.bfloat16, ml_dtypes.bfloat16),
}


# ---------------------------------------------------------------------------
# Workarounds for the pinned walrus build's 1-wait-per-instruction cap.
# Tile's tail drain stuffs every outstanding sem wait onto one Drain, and
# self-loading fp32/fp32r matmuls can carry >1 wait with no Ldweights to
# spill to. Split both across extra same-engine instructions.
def _patched_drain_and_barrier(self, tick_clock, wait_clock):
    # NOTE: the final drain+barrier is REQUIRED — removing it entirely
    # makes the untraced NRT execution path fail with
    # NRT_EXEC_UNIT_UNRECOVERABLE (verified 2026-08-08), even though the
    # traced path tolerates it.  Only clear_and_free_semaphores (redundant
    # with the NRT postamble's blanket sem reset) is skipped.
    drain_inst = self.nc.sync.drain()
    wait_clock.add_sem_waits(
        drain_inst.ins, ScopedClock({None: tick_clock.global_clock})
    )
    si = drain_inst.ins.sync_info
    waits = list(si.on_wait)
    if len(waits) > 1:
        drain_inst.ins.sync_info = mybir.SyncInfo(
            on_wait=[waits[0]], on_update=list(si.on_update)
        )
        # spread the remaining waits across all engines so they resolve in
        # parallel; the barrier below joins them.
        engs = [self.nc.vector, self.nc.scalar, self.nc.gpsimd,
                self.nc.tensor, self.nc.sync]
        for i, w in enumerate(waits[1:]):
            d2 = engs[i % len(engs)].drain()
            d2.ins.sync_info = mybir.SyncInfo(on_wait=[w], on_update=[])
    self.nc.all_engine_barrier()
    assert self.sems is not None
    popped = self.nc._tile_sem_poison_stack.pop()
    assert popped is self._sem_poison


EXTRA_WALRUS_FLAGS = []


def _apply_tile_patch():
    TileContext._drain_and_barrier = _patched_drain_and_barrier
    if EXTRA_WALRUS_FLAGS:
        import concourse.bass_utils as bu
        orig = bu.bir_verify_and_optimise

        def patched(tmpdir, inp="bir.json", outp="file.neff", arch=None, *,
                    dve_root=None):
            import concourse.bass_utils as bu2
            real_run = bu2.run_command

            def run_with_flags(cmd, **kw):
                if cmd and str(cmd[0]).endswith("walrus_driver"):
                    cmd = list(cmd) + EXTRA_WALRUS_FLAGS
                return real_run(cmd, **kw)

            bu2.run_command = run_with_flags
            try:
                return orig(tmpdir, inp, outp, arch, dve_root=dve_root)
            finally:
                bu2.run_command = real_run

        bu.bir_verify_and_optimise = patched


def _install_profile_shim():
    """Make trace=True work in this container: provide the missing
    antenv.axon_hooks module (NTFF capture via ctypes into libaxon_pjrt.so)
    and skip the bucket upload of artifacts."""
    import sys, types, ctypes, contextlib
    import concourse.bass_utils as bu
    bu.upload_artifacts = lambda tmpdir: tmpdir
    try:
        from antenv.axon_hooks import get_axon_ntff_profile_hook  # noqa
        return
    except ImportError:
        pass
    so_path = "/opt/axon/libaxon_pjrt.so"
    lib = ctypes.CDLL(so_path)
    if not hasattr(lib, "axon_start_nrt_profile"):
        return
    lib.axon_start_nrt_profile.argtypes = [
        ctypes.POINTER(ctypes.c_int64), ctypes.c_size_t]
    lib.axon_start_nrt_profile.restype = ctypes.c_int64
    lib.axon_stop_nrt_profile.argtypes = [ctypes.c_char_p]
    lib.axon_stop_nrt_profile.restype = ctypes.c_int64

    @contextlib.contextmanager
    def _hook(output_dir, device_ids):
        import jax
        jax.devices()
        if device_ids:
            ids = (ctypes.c_int64 * len(device_ids))(*device_ids)
            rc = lib.axon_start_nrt_profile(ids, len(device_ids))
        else:
            rc = lib.axon_start_nrt_profile(None, 0)
        if rc != 0:
            raise RuntimeError(f"axon_start_nrt_profile rc={rc}")
        try:
            yield
        finally:
            n = lib.axon_stop_nrt_profile(str(output_dir).encode())
            print(f"profile: {n} file(s) written to {output_dir}")

    mod = types.ModuleType("antenv.axon_hooks")
    mod.get_axon_ntff_profile_hook = lambda: _hook
    mod.set_axon_ntff_profile_hook = lambda h: None
    sys.modules["antenv.axon_hooks"] = mod
    import antenv
    antenv.axon_hooks = mod


def _hoist_excess_waits(nc):
    """Any non-EventSemaphore instruction may carry at most 1 sem wait on
    this walrus build; move extras onto inserted same-engine NoOps."""
    for fn in nc.m.functions:
        for bb in fn.blocks:
            insts = bb.instructions
            newlist = []
            changed = False
            for inst in insts:
                si = inst.sync_info
                cap = 2 if isinstance(inst, mybir.InstEventSemaphore) else 1
                if si is not None and len(si.on_wait) > cap:
                    waits = list(si.on_wait)
                    for i, w in enumerate(waits[cap:]):
                        newlist.append(mybir.InstNoOp(
                            name=f"{inst.name}_waitnop{i}",
                            engine=inst.engine,
                            bass_nofuse=True,
                            sync_info=mybir.SyncInfo(on_wait=[w], on_update=[]),
                        ))
                    inst.sync_info = mybir.SyncInfo(
                        on_wait=waits[:cap], on_update=list(si.on_update))
                    changed = True
                newlist.append(inst)
            if changed:
                insts.clear()
                insts.extend(newlist)


# ---------------------------------------------------------------------------
def _build_nc(mode):
    dt_in, _ = _DT[mode]
    f32 = mybir.dt.float32
    dt_out = dt_in if mode == "bf16" else f32
    nc = bass.Bass()
    # partition-major, batch-tile-major staging: every DMA below is
    # contiguous per SBUF partition (128 fat descriptors instead of
    # thousands of 512B-1KB ones), minimizing DGE latency and ring pressure
    xT_d = nc.dram_tensor(
        "xT", [NBT, 128, NHALO * 2, BT], dt_in, kind="ExternalInput")
    w_d = nc.dram_tensor(
        "w", [128, BPC * 3 * 2, P], dt_in, kind="ExternalInput")
    o_d = nc.dram_tensor(
        "o", [NBT, 128, 16, BT], dt_out, kind="ExternalOutput")

    with TileContext(nc) as tc:
        with tc.tile_pool(name="wpool", bufs=1) as wpool, \
             tc.tile_pool(name="xpool", bufs=2) as xpool, \
             tc.tile_pool(name="opool", bufs=2) as opool, \
             tc.tile_pool(name="pspool", bufs=8, space="PSUM") as pspool:
            w_sb = wpool.tile([128, BPC * 3 * 2, P], dt_in)

            def w_load(j0, j1):
                nc.sync.dma_start(
                    out=w_sb[:, j0:j1, :], in_=w_d[:, j0:j1, :])

            xt0 = xpool.tile([128, NHALO * 2, BT], dt_in, tag="x")

            def x0_load(b0, b1):
                nc.sync.dma_start(
                    out=xt0[:, 2 * b0:2 * b1, :],
                    in_=xT_d[0, :, 2 * b0:2 * b1, :])

            # head: interleave w/x chunks in consumption order; few fat
            # chunks — each dma_start costs ~0.7us of sync-engine issue
            # time, so over-chunking delays the bulk loads
            w_load(0, 2)
            x0_load(0, 1)
            w_load(2, 6)
            x0_load(1, 3)
            w_load(6, 12)
            x0_load(3, 10)
            w_load(12, 24)
            w_load(24, 48)

            for bt in range(NBT):
                if bt == 0:
                    xt = xt0
                else:
                    xt = xpool.tile([128, NHALO * 2, BT], dt_in, tag="x")
                    nc.sync.dma_start(out=xt, in_=xT_d[bt])
                for h in range(2):          # out-group halves
                    ot = opool.tile([128, 8, BT], dt_out, tag="o")
                    last = (bt == NBT - 1 and h == 1)
                    # chunked output DMAs: fire as soon as slots fill; the
                    # final group drains in 2-slot pieces (csz=1 tested 3x:
                    # totals 187.7/189.4/191.0, tail segment 3.35us vs ~3.4
                    # estimated baseline — inconclusive, kept the 10-sample
                    # verified incumbent)
                    csz = 2 if last else 4
                    for li in range(h * 4, h * 4 + 4):
                        for qh in range(2):
                            ps = pspool.tile([128, BT], f32, tag="ps")
                            for s in range(3):
                                for kh in range(2):
                                    nc.tensor.matmul(
                                        ps,
                                        w_sb[:, (li * 3 + s) * 2 + kh,
                                             qh * 128:(qh + 1) * 128],
                                        xt[:, (li + s) * 2 + kh, :],
                                        start=(s == 0 and kh == 0),
                                        stop=(s == 2 and kh == 1),
                                    )
                            nc.vector.tensor_copy(
                                out=ot[:, (li * 2 + qh) - h * 8, :], in_=ps)
                            slot = (li - h * 4) * 2 + qh + 1
                            if slot % csz == 0:
                                nc.sync.dma_start(
                                    out=o_d[bt, :, h * 8 + slot - csz:
                                            h * 8 + slot, :],
                                    in_=ot[:, slot - csz:slot, :],
                                )
    _hoist_excess_waits(nc)
    return nc


def _host_prep(x, Wd, Wu, Wl, Wtr, Wbl, np_dt):
    x = np.asarray(x, np.float32)
    Wd, Wu, Wl = np.asarray(Wd, np.float32), np.asarray(Wu, np.float32), np.asarray(Wl, np.float32)
    Wtr, Wbl = np.asarray(Wtr, np.float32), np.asarray(Wbl, np.float32)

    xT = np.ascontiguousarray(x.T)                       # [M*P, B]
    A = np.concatenate([Wtr[None], Wl], axis=0)          # weight applied to x_{i-1}
    Bst = Wd                                             # weight applied to x_i
    C = np.concatenate([Wu, Wbl[None]], axis=0)          # weight applied to x_{i+1}
    WT = np.stack([A, Bst, C], axis=1)                   # [64, 3, q, p]
    WT = np.ascontiguousarray(WT.transpose(0, 1, 3, 2))  # [64, 3, p, q]

    in_maps = []
    for c in range(NCORES):
        lo = (8 * c - 1) * P
        hi = (8 * c + 9) * P
        if lo < 0:
            xc = np.concatenate([xT[lo:], xT[:hi]], axis=0)
        elif hi > M * P:
            xc = np.concatenate([xT[lo:], xT[:hi - M * P]], axis=0)
        else:
            xc = xT[lo:hi]
        xc = np.asarray(xc, dtype=np_dt)                 # [2560, 4096]
        # partition-major, bt-major: xp[bt, p, t, b] = xc[t*128+p, bt*BT+b]
        xp = np.ascontiguousarray(
            xc.reshape(NHALO * 2, 128, NBT, BT).transpose(2, 1, 0, 3))
        wc = WT[8 * c:8 * c + 8].reshape(BPC, 3, 2, 128, P)
        wc = np.asarray(wc.reshape(BPC * 3 * 2, 128, P), dtype=np_dt)
        wp = np.ascontiguousarray(wc.transpose(1, 0, 2))  # [128, 48, 256]
        in_maps.append({"xT": xp, "w": wp})
    return in_maps


def kernel(x, Wd, Wu, Wl, Wtr, Wbl):
    global LAST_EXEC_NS
    _apply_tile_patch()
    if TRACE:
        try:
            _install_profile_shim()
        except Exception as e:
            print(f"profile shim failed ({e}); running without trace")
    dt_in, np_dt = _DT[MODE]
    nc = _build_nc(MODE)
    in_maps = _host_prep(x, Wd, Wu, Wl, Wtr, Wbl, np_dt)
    res = run_bass_kernel_spmd(
        nc, in_maps, core_ids=list(range(NCORES)), trace=TRACE)
    LAST_EXEC_NS = res.exec_time_ns
    if TRACE and REPEATS > 1:
        global ALL_EXEC_NS
        ALL_EXEC_NS = [res.exec_time_ns]
        for _ in range(REPEATS - 1):
            r2 = run_bass_kernel_spmd(
                nc, in_maps, core_ids=list(range(NCORES)), trace=True)
            ALL_EXEC_NS.append(r2.exec_time_ns)
        LAST_EXEC_NS = min(t for t in ALL_EXEC_NS if t)
    # o is [NBT, 128, 16, BT] per core: o[bt, p, j, b] = outT[j*128+p, bt*BT+b]
    outT = np.concatenate(
        [np.asarray(res.results[c]["o"]).transpose(2, 1, 0, 3).reshape(
            BPC * P, B) for c in range(NCORES)], axis=0)
    return np.ascontiguousarray(outT.T, dtype=np.float32)  # [B, M*P] float32

